# revision 27
# baseline (speedup 1.0000x reference)
"""Trainium2 Bass kernel for nn_AdaptiveValuesMetadataAttention.

Shapes (hardcoded from the problem spec):
  values   [1, 8, 512, 256]  metadata [1, 8, 512, 64]
  w_meta_outer [64, 512]  w_qkv [256, 768]  w_meta_inner [64, 512]
  w_out [256, 256]  b_out [256]

Host does the data-dependent top-3 window selection + gather; each of the
8 NeuronCores computes one source's inner fused attention (queries =
window slot-0 tokens, keys/values = all 3*512 window tokens).

Layout: scores for head pair t accumulate in psA/psB [128kv, 1536=3x512q]
PSUM tiles; exp goes to bf16 SBUF (ScalarE exact exp for the A band, DVE
custom squared-cubic exp for the B band); attn@V runs col-tiled so the A
band (psum parts 0:33) and B band (64:97) matmuls execute concurrently.
Denominators ride along as a ones-column in V; tails do one DVE
reciprocal straight off PSUM, gpsimd partition-broadcasts, and two DVE
muls.  Output projection is 8 K=128 matmuls against zero-padded weights.
"""

import numpy as np

B, S, N, DV, DM = 1, 8, 512, 256, 64
INNER, H, WS = 256, 8, 3
DH = INNER // H          # 32
W = WS * N               # 1536 kv tokens per window
SCALE = DH ** -0.5
VP = 264                 # packed V width: 8 heads x (32 v + 1 ones)

# Every exp tile is split across both engines: ScalarE computes the
# first SF columns as K*exp(x/sqrt(32)) via bias=ln(K); the DVE computes
# the rest with the squared-cubic poly (which natively produces K*exp).
# The common K cancels in the softmax normalization.
SF = 832
LNK = 18.18385213360405
# monic cubic q=((x+C0)x+C1)x+C2 with q^2 ~ K*exp(x/sqrt(32)) on
# |x|<=7.5 (observed max |score| 5.4); K cancels per-head in softmax.
EC0, EC1, EC2 = 35.92931248855501, 786.8150028483795, 8875.074011357667

_CACHE = {}


def _register_dve_exp():
    """Register the custom DVE op EXP_SQPOLY_ANT (idempotent)."""
    from concourse import dve_ops
    from concourse.dve_spec import Spec, Src0, C0, C1, C2, lower, sq
    from concourse.dve_uop import DveOpSpec

    for op in dve_ops.OPS:
        if op.name == "EXP_SQPOLY_ANT":
            return op
    q = ((Src0 + C0) * Src0 + C1) * Src0 + C2
    body = sq(q)

    def ref(in0, in1, c0, c1, c2):
        qq = ((in0.astype(np.float32) + np.float32(c0)) * in0
              + np.float32(c1)) * in0 + np.float32(c2)
        return qq * qq

    spec = Spec(body=body, reference=ref)
    row = max(dve_ops._SUB_OPCODE_FOR_NAME.values()) + 1
    assert row < 0x20, row
    dve_ops._SUB_OPCODE_FOR_NAME["EXP_SQPOLY_ANT"] = row
    shas = {}
    for ver in ("v3", "v4"):
        s = DveOpSpec(name="EXP_SQPOLY_ANT", opcode=row,
                      uops=lower(spec, ver=ver), rd1_en=False)
        shas[ver] = s.sha(ver)
    op = dve_ops.DveOp("EXP_SQPOLY_ANT", spec, subdim=False, uops_sha=shas)
    dve_ops.OPS.append(op)
    dve_ops.CUSTOM_DVE_SPECS["EXP_SQPOLY_ANT"] = spec
    return op


def _host_top_idx(values, metadata, w_meta_outer):
    meta_mean = metadata.mean(axis=2)                        # [B,S,DM]
    qk = meta_mean @ w_meta_outer                            # [B,S,2*INNER]
    qm = np.clip(qk[..., :INNER], -5, 5)
    km = np.clip(qk[..., INNER:], -5, 5)
    dots = np.einsum('bqd,bkd->bqk', qm, km) * (INNER ** -0.5)
    m = dots.max(-1, keepdims=True)
    e = np.exp(dots - m)
    attn = e / e.sum(-1, keepdims=True)
    attn = attn + 2.0 * np.eye(S, dtype=attn.dtype)
    return np.argsort(-attn, axis=-1, kind='stable')[..., :WS]  # [B,S,WS]


def _build_bass():
    import concourse.bass as bass  # noqa: F401
    import concourse.tile as tile
    from concourse import bacc, mybir

    F32 = mybir.dt.float32
    BF16 = mybir.dt.bfloat16
    EXP = mybir.ActivationFunctionType.Exp
    COPY = mybir.ActivationFunctionType.Copy
    MIN = mybir.AluOpType.min
    MAX = mybir.AluOpType.max

    exp_op = _register_dve_exp()
    nc = bacc.Bacc(None, target_bir_lowering=False)

    # fused contiguous dram tensors: one dma_start each (issue overhead
    # is ~650ns per DMA on the issuing queue; a single DMA fans out over
    # 16 SDMA engines, so fewer+larger+contiguous wins).
    kvtb = [nc.dram_tensor(f"kvtb{b2}", [128, 1024], BF16,
                           kind="ExternalInput") for b2 in range(3)]
    kvmT2 = nc.dram_tensor("kvmT2", [128, W], BF16, kind="ExternalInput")
    wc12q = nc.dram_tensor("wc12q", [128, 1024], BF16, kind="ExternalInput")
    wc12k = nc.dram_tensor("wc12k", [128, 1024], BF16, kind="ExternalInput")
    wc3d = nc.dram_tensor("wc3d", [128, 1024], BF16, kind="ExternalInput")
    wvd = nc.dram_tensor("wvd", [128, 2 * VP], BF16, kind="ExternalInput")
    wop = nc.dram_tensor("wop", [128, 4 * DV], BF16, kind="ExternalInput")
    bo = nc.dram_tensor("bo", [128, 2], F32, kind="ExternalInput")
    out = nc.dram_tensor("out", [DV, N], BF16, kind="ExternalOutput")

    with tile.TileContext(nc) as tc:
        with (
            tc.tile_pool(name="w", bufs=1) as wp,
            tc.tile_pool(name="big", bufs=1) as bigp,
            tc.tile_pool(name="expp", bufs=6) as expp,
            tc.tile_pool(name="tails", bufs=4) as tailsb,
        ):
            # ---- persistent SBUF: inputs + weights --------------------
            # per-512-block kv tiles so consumers only wait on the DMAs
            # they need (deps are whole-tile); each tile loads with ONE
            # contiguous dma_start.
            kvT_sb = [wp.tile([128, 1024], BF16, tag=f"kvTb{b2}",
                              name=f"kvTb{b2}") for b2 in range(3)]
            wv_sb = wp.tile([128, 2 * VP], BF16, tag="wv")
            wc12q_sb = wp.tile([128, 1024], BF16, tag="wc12q")
            wc12k_sb = wp.tile([128, 1024], BF16, tag="wc12k")
            wc3_sb = wp.tile([128, 1024], BF16, tag="wc3")
            kvmT_sb = wp.tile([128, W], BF16, tag="kvmT")
            wop_sb = wp.tile([128, 4 * DV], BF16, tag="wop")
            b_sb = wp.tile([128, 2], F32, tag="b")

            # sync ring: kvtb0, kvtb2, wc12k, bo
            nc.sync.dma_start(out=kvT_sb[0][:], in_=kvtb[0][:])
            nc.sync.dma_start(out=kvT_sb[2][:], in_=kvtb[2][:])
            nc.sync.dma_start(out=wc12k_sb[:], in_=wc12k[:])
            nc.sync.dma_start(out=b_sb[:], in_=bo[:])
            # gpsimd ring: kvtb1, wc3d, wop
            nc.gpsimd.dma_start(out=kvT_sb[1][:], in_=kvtb[1][:])
            nc.gpsimd.dma_start(out=wc3_sb[:], in_=wc3d[:])
            nc.gpsimd.dma_start(out=wop_sb[:], in_=wop[:])
            # scalar ring: wv, wc12q, kvmT2
            nc.scalar.dma_start(out=wv_sb[:], in_=wvd[:])
            nc.scalar.dma_start(out=wc12q_sb[:], in_=wc12q[:])
            nc.scalar.dma_start(out=kvmT_sb[:], in_=kvmT2[:])
            ones_sb = wp.tile([128, 512], BF16, tag="ones")
            nc.vector.memset(ones_sb[:], 1.0)
            ones32_sb = wp.tile([128, 32], F32, tag="ones32")
            nc.vector.memset(ones32_sb[:], 1.0)
            lnk_sb = wp.tile([128, 1], F32, tag="lnk")
            nc.vector.memset(lnk_sb[:], LNK)

            # ---- persistent SBUF: projection outputs ------------------
            Qz_sb = [[bigp.tile([128, N], BF16, tag=f"Qz{t}{u}",
                                name=f"Qz{t}{u}") for u in range(2)]
                     for t in range(4)]
            for t in range(4):
                nc.gpsimd.memset(Qz_sb[t][0][64:128, :], 0.0)
                nc.gpsimd.memset(Qz_sb[t][1][0:64, :], 0.0)
            KcatT_sb = [bigp.tile([128, W], BF16, tag=f"Kc{t}", name=f"Kc{t}")
                        for t in range(4)]
            V_sb = [bigp.tile([128, VP], BF16, tag=f"V{c}", name=f"V{c}")
                    for c in range(12)]
            # per-pair normalized outputs: head 2t at parts 0:32,
            # head 2t+1 at parts 64:96; other rows zeroed once (the
            # out-proj contracts K=128 against zero-padded weights).
            OT_sb = [bigp.tile([128, N], BF16, tag=f"OT{t}",
                               name=f"OT{t}") for t in range(4)]
            for t in range(4):
                nc.gpsimd.memset(OT_sb[t][32:64, :], 0.0)
                nc.gpsimd.memset(OT_sb[t][96:128, :], 0.0)

            def clip_copy(dst, src):
                nc.vector.tensor_scalar(dst, src, 5.0, -5.0, MIN, MAX)

            # ---- phase 0: PE warm-up during the input DMA wait --------
            with tc.tile_pool(name="warm", bufs=1, space="PSUM") as warmp:
                wps = warmp.tile([32, 512], F32, tag="warm", name="wps")
                for i in range(7):
                    nc.tensor.matmul(wps[:], ones_sb[0:64, 0:32],
                                     ones_sb[0:64, :])

            # ---- phase 1: projections ---------------------------------
            with tc.tile_pool(name="proj", bufs=6, space="PSUM") as projp:
                def emit_v():
                    for c in range(12):
                        blk2, lo = c // 4, 128 * (c % 4)
                        cs = slice(lo, lo + 128)
                        ps = projp.tile([128, VP], F32, tag="proj",
                                        name="psv")
                        nc.tensor.matmul(ps[:], kvT_sb[blk2][:, cs],
                                         wv_sb[:, 0:VP],
                                         start=True, stop=False)
                        nc.tensor.matmul(ps[:], kvT_sb[blk2][:, 512:1024][:, cs],
                                         wv_sb[:, VP:],
                                         start=False, stop=True)
                        nc.vector.tensor_copy(V_sb[c][:], ps[:])
                        v33 = V_sb[c][:].rearrange("p (v w) -> p v w", w=33)
                        nc.vector.memset(v33[:, :, 32:33], 1.0)

                def emit_q():
                    for ta, tb in ((0, 1), (2, 3)):
                        pss = {}
                        for t in (ta, tb):
                            cs = slice(128 * t, 128 * (t + 1))
                            ps = projp.tile([128, N], F32, tag="proj",
                                            name="psq")
                            pss[t] = ps
                            nc.tensor.matmul(ps[:], wc12q_sb[:, 0:512][:, cs],
                                             kvT_sb[0][:, 0:512],
                                             start=True, stop=False)
                            nc.tensor.matmul(ps[:], wc12q_sb[:, 512:1024][:, cs],
                                             kvT_sb[0][:, 512:1024],
                                             start=False, stop=False)
                        # K=64 meta matmuls at disjoint row groups: concurrent
                        nc.tensor.matmul(pss[ta][:],
                                         wc3_sb[0:64, 512:1024][:, 128*ta:128*ta+128],
                                         kvmT_sb[0:64, 0:N],
                                         start=False, stop=True,
                                         tile_position=(0, 0))
                        nc.tensor.matmul(pss[tb][:],
                                         wc3_sb[64:128, 512:1024][:, 128*tb:128*tb+128],
                                         kvmT_sb[64:128, 0:N],
                                         start=False, stop=True,
                                         tile_position=(64, 0))
                        for t in (ta, tb):
                            clip_copy(Qz_sb[t][0][0:64, :], pss[t][0:64, :])
                            clip_copy(Qz_sb[t][1][64:128, :], pss[t][64:128, :])

                def emit_k():
                    for bk in range(3):
                        fs = slice(512 * bk, 512 * (bk + 1))
                        for ta, tb in ((0, 1), (2, 3)):
                            pss = {}
                            for t in (ta, tb):
                                cs = slice(128 * t, 128 * (t + 1))
                                ps = projp.tile([128, N], F32, tag="proj",
                                                name="psk")
                                pss[t] = ps
                                nc.tensor.matmul(ps[:], wc12k_sb[:, 0:512][:, cs],
                                                 kvT_sb[bk][:, 0:512],
                                                 start=True, stop=False)
                                nc.tensor.matmul(ps[:], wc12k_sb[:, 512:1024][:, cs],
                                                 kvT_sb[bk][:, 512:1024],
                                                 start=False, stop=False)
                            nc.tensor.matmul(pss[ta][:],
                                             wc3_sb[0:64, 0:512][:, 128*ta:128*ta+128],
                                             kvmT_sb[0:64, fs],
                                             start=False, stop=True,
                                             tile_position=(0, 0))
                            nc.tensor.matmul(pss[tb][:],
                                             wc3_sb[64:128, 0:512][:, 128*tb:128*tb+128],
                                             kvmT_sb[64:128, fs],
                                             start=False, stop=True,
                                             tile_position=(64, 0))
                            for t in (ta, tb):
                                if (bk + t) % 2 == 0:
                                    clip_copy(KcatT_sb[t][:, fs], pss[t][:])
                                else:
                                    nc.scalar.activation(KcatT_sb[t][:, fs],
                                                         pss[t][:], COPY)

                emit_v(); emit_q(); emit_k()

            # ---- phase 2: software-pipelined attention ----------------
            # Per (t, blk) item: emit scores(k) then attn@V(k-1), so the
            # exp latency of block k hides under the next block's score
            # matmuls.  Tail work for pair t is split into pieces emitted
            # one and two items later (plenty of slack: outps slot t is
            # only recycled at t+2).
            NBLK = 4
            with (
                tc.tile_pool(name="sc", bufs=2, space="PSUM") as scp,
                tc.tile_pool(name="tail", bufs=2, space="PSUM") as tailp,
            ):
                items = [(t, b) for t in range(4) for b in range(NBLK)]
                n_items = len(items)
                pieces = [[] for _ in range(n_items + 5)]
                ebuf = {}
                outps_by_t = {}

                def emit_S(i):
                    t, b = items[i]
                    psA = scp.tile([128, 1536], F32, tag="sc", name="psA")
                    psB = scp.tile([128, 1536], F32, tag="sc", name="psB")
                    for j in range(3):
                        c = 3 * b + j
                        cs = slice(128 * c, 128 * (c + 1))
                        js = slice(512 * j, 512 * (j + 1))
                        nc.tensor.matmul(psA[:, js], KcatT_sb[t][:, cs],
                                         Qz_sb[t][0][:])
                        nc.tensor.matmul(psB[:, js], KcatT_sb[t][:, cs],
                                         Qz_sb[t][1][:])
                    eA = expp.tile([128, 1536], BF16, tag="exp", name="eA")
                    eB = expp.tile([128, 1536], BF16, tag="exp", name="eB")
                    nc.scalar.activation(eA[:, 0:SF], psA[:, 0:SF], EXP,
                                         bias=lnk_sb[:], scale=SCALE)
                    nc.vector._custom_dve(exp_op, out=eA[:, SF:],
                                          in0=psA[:, SF:], s0=EC0,
                                          s1=EC1, imm2=EC2)
                    nc.scalar.activation(eB[:, 0:SF], psB[:, 0:SF], EXP,
                                         bias=lnk_sb[:], scale=SCALE)
                    nc.vector._custom_dve(exp_op, out=eB[:, SF:],
                                          in0=psB[:, SF:], s0=EC0,
                                          s1=EC1, imm2=EC2)
                    ebuf[i] = (eA, eB)

                def emit_AV(i):
                    t, b = items[i]
                    if b == 0:
                        outps_by_t[t] = tailp.tile([128, N], F32, tag="outps",
                                                   name="outps")
                    outps = outps_by_t[t]
                    eA, eB = ebuf.pop(i)
                    for j in range(3):
                        c = 3 * b + j
                        js = slice(512 * j, 512 * (j + 1))
                        nc.tensor.matmul(
                            outps[0:33, :], V_sb[c][:, 66 * t:66 * t + 33],
                            eA[:, js], start=(c == 0), stop=(c == 11),
                            skip_group_check=True)
                        nc.tensor.matmul(
                            outps[64:97, :], V_sb[c][:, 66 * t + 33:66 * t + 66],
                            eB[:, js], start=(c == 0), stop=(c == 11),
                            skip_group_check=True)
                    if b == NBLK - 1:
                        # tail piece 0 (now): reciprocal straight off PSUM
                        # (rows 32 / 96 hold denominators via V's ones col)
                        rcpS = tailsb.tile([128, N], F32, tag="rcp",
                                           name="rcpS")
                        nc.vector.reciprocal_approx_fast(out=rcpS[0:97, :],
                                                         in_=outps[0:97, :])
                        cell = {}

                        def piece_bcast(t=t, rcpS=rcpS, cell=cell):
                            T1 = scp.tile([128, N], F32, tag="sc", name="T1")
                            nc.tensor.matmul(T1[0:32, :], ones32_sb[32:33, :],
                                             rcpS[32:33, :],
                                             tile_position=(32, 0))
                            nc.tensor.matmul(T1[64:96, :], ones32_sb[96:97, :],
                                             rcpS[96:97, :],
                                             tile_position=(96, 64))
                            rbS = tailsb.tile([128, N], F32, tag="rb",
                                              name="rbS")
                            nc.scalar.activation(rbS[0:97, :], T1[0:97, :],
                                                 COPY)
                            cell["rbS"] = rbS

                        def piece_mulA(t=t, outps=outps, cell=cell):
                            nc.vector.tensor_mul(OT_sb[t][0:32, :],
                                                 outps[0:32, :],
                                                 cell["rbS"][0:32, :])

                        def piece_mulB(t=t, outps=outps, cell=cell):
                            nc.vector.tensor_mul(OT_sb[t][64:96, :],
                                                 outps[64:96, :],
                                                 cell["rbS"][64:96, :])

                        pieces[i + 2].append(piece_bcast)
                        pieces[i + 3].append(piece_mulA)
                        pieces[i + 4].append(piece_mulB)

                for i in range(n_items + 1):
                    if i < n_items:
                        emit_S(i)
                    if i >= 1:
                        emit_AV(i - 1)
                    for fn in pieces[i]:
                        fn()
                for k in range(n_items + 1, n_items + 5):
                    for fn in pieces[k]:
                        fn()

            # ---- phase 3: output projection + bias --------------------
            with tc.tile_pool(name="fin", bufs=2, space="PSUM") as finp:
                for d in range(2):
                    sl = slice(128 * d, 128 * (d + 1))
                    ops = finp.tile([128, N], F32, tag="fin", name="ops")
                    for t in range(4):
                        ws = slice(256 * t + 128 * d, 256 * t + 128 * (d + 1))
                        nc.tensor.matmul(ops[:], wop_sb[:, ws], OT_sb[t][:],
                                         start=(t == 0), stop=(t == 3))
                    fin = tailsb.tile([128, N], BF16, tag="fin", name="fin")
                    nc.vector.tensor_scalar_add(fin[:], ops[:], b_sb[:, d:d + 1])
                    if d == 0:
                        nc.sync.dma_start(out=out[sl, :], in_=fin[:])
                    else:
                        nc.gpsimd.dma_start(out=out[sl, :], in_=fin[:])

    nc.compile()
    return nc


def _get_nc():
    if "nc" not in _CACHE:
        _CACHE["nc"] = _build_bass()
    return _CACHE["nc"]


def _pack_weights(w_qkv, w_meta_inner, w_out, b_out):
    import ml_dtypes
    bf = ml_dtypes.bfloat16
    f = np.float32
    wq = w_qkv[:, :INNER]
    wk = w_qkv[:, INNER:2 * INNER]
    wv = w_qkv[:, 2 * INNER:]
    wmq = w_meta_inner[:, :INNER]
    wmk = w_meta_inner[:, INNER:]

    def cat_pack(wp_, wm_):
        p1 = np.zeros((128, 512), dtype=np.float32)
        p2 = np.zeros((128, 512), dtype=np.float32)
        p3 = np.zeros((64, 512), dtype=np.float32)
        for t in range(4):
            a, b2 = 2 * t, 2 * t + 1
            c0 = 128 * t
            p1[:, c0 + 0:c0 + 32] = wp_[0:128, 32 * a:32 * a + 32]
            p2[:, c0 + 0:c0 + 32] = wp_[128:256, 32 * a:32 * a + 32]
            p3[:, c0 + 32:c0 + 64] = wm_[:, 32 * a:32 * a + 32]
            p1[:, c0 + 64:c0 + 96] = wp_[0:128, 32 * b2:32 * b2 + 32]
            p2[:, c0 + 64:c0 + 96] = wp_[128:256, 32 * b2:32 * b2 + 32]
            p3[:, c0 + 96:c0 + 128] = wm_[:, 32 * b2:32 * b2 + 32]
        return p1, p2, p3

    k1, k2, k3 = cat_pack(wk, wmk)
    q1, q2, q3 = cat_pack(wq, wmq)
    wc12q = np.ascontiguousarray(
        np.concatenate([q1, q2], axis=1)).astype(bf)           # [128, 1024]
    wc12k = np.ascontiguousarray(
        np.concatenate([k1, k2], axis=1)).astype(bf)           # [128, 1024]
    wc3h = np.concatenate([k3, q3], axis=1)                    # [64, 1024]
    wc3d = np.ascontiguousarray(
        np.concatenate([wc3h, wc3h], axis=0)).astype(bf)       # [128, 1024]

    # V: per pair t a 66-col band [32 va | 0(ones slot) | 32 vb | 0(ones)]
    wvp = np.zeros((256, VP), dtype=np.float32)
    for t in range(4):
        wvp[:, 66 * t + 0:66 * t + 32] = wv[:, 64 * t:64 * t + 32]
        wvp[:, 66 * t + 33:66 * t + 65] = wv[:, 64 * t + 32:64 * t + 64]
    wvd = np.ascontiguousarray(
        np.concatenate([wvp[0:128, :], wvp[128:256, :]], axis=1)).astype(bf)

    # out-proj: per pair t a 256-col band; head 2t rows 0:32, head 2t+1
    # rows 64:96, rest zero (junk OT rows hit zero weights).
    wopk = np.zeros((128, 4 * DV), dtype=np.float32)
    for t in range(4):
        wopk[0:32, 256 * t:256 * t + 256] = w_out[64 * t:64 * t + 32, :]
        wopk[64:96, 256 * t:256 * t + 256] = w_out[64 * t + 32:64 * t + 64, :]
    wopk = np.ascontiguousarray(wopk).astype(bf)

    bo = np.ascontiguousarray(
        np.stack([b_out[0:128], b_out[128:256]], axis=1), dtype=f)
    return {"wc12q": wc12q, "wc12k": wc12k, "wc3d": wc3d,
            "wvd": wvd, "wop": wopk, "bo": bo}


def build_in_maps(values, metadata, w_qkv, w_meta_inner, w_out, b_out, top_idx):
    import ml_dtypes
    bf = ml_dtypes.bfloat16
    shared = _pack_weights(w_qkv, w_meta_inner, w_out, b_out)
    in_maps = []
    for s in range(S):
        idx = top_idx[0, s]
        kvT = values[0, idx].reshape(W, DV).T.astype(bf)       # [256, 1536]
        kvmT = metadata[0, idx].reshape(W, DM).T.astype(bf)    # [64, 1536]
        im = dict(shared)
        im["kvmT2"] = np.ascontiguousarray(
            np.concatenate([kvmT, kvmT], axis=0))              # [128, 1536]
        for b2 in range(3):
            im[f"kvtb{b2}"] = np.ascontiguousarray(np.concatenate(
                [kvT[0:128, 512 * b2:512 * (b2 + 1)],
                 kvT[128:256, 512 * b2:512 * (b2 + 1)]], axis=1))
        in_maps.append(im)
    return in_maps


def kernel(values, metadata, w_meta_outer, w_qkv, w_meta_inner, w_out, b_out,
           _trace=False):
    from concourse.bass_utils import run_bass_kernel_spmd

    values = np.asarray(values, dtype=np.float32)
    metadata = np.asarray(metadata, dtype=np.float32)
    w_meta_outer = np.asarray(w_meta_outer, dtype=np.float32)
    w_qkv = np.asarray(w_qkv, dtype=np.float32)
    w_meta_inner = np.asarray(w_meta_inner, dtype=np.float32)
    w_out = np.asarray(w_out, dtype=np.float32)
    b_out = np.asarray(b_out, dtype=np.float32)

    top_idx = _host_top_idx(values, metadata, w_meta_outer)
    assert (top_idx[0, :, 0] == np.arange(S)).all(), top_idx

    in_maps = build_in_maps(values, metadata, w_qkv, w_meta_inner, w_out,
                            b_out, top_idx)
    nc = _get_nc()
    res = run_bass_kernel_spmd(nc, in_maps, core_ids=list(range(S)),
                               trace=_trace)
    out = np.stack([res.results[s]["out"].T.astype(np.float32)
                    for s in range(S)], axis=0)
    _CACHE["last_result"] = res
    return out.reshape(B, S, N, DV)
